# revision 1
# baseline (speedup 1.0000x reference)
# Bass/Tile TRN2 kernel for nn_Conv1D_style: out = ((x * (cluster@style_L)) @ weight) * (cluster@style_R)
#
# Sharding: data-parallel over the batch dim. Each of the 8 cores gets a
# 1024-row slice of x/cluster and a full (replicated) weight/style_L/style_R.
#
# Per-core plan (M=1024 batch, K=4096 din, N=4096 dout), all matmuls bf16
# with fp32 PSUM accumulation:
#   aT[k] = xT[k] * (style_L[:, kslice].T @ clusterT)  -> bf16, SBUF-resident.
#   y[m,n] = sum_k aT[k][:, mslice].T @ W[k, nslice]   (32 accumulating MMs)
#   out[m,n] = y[m,n] * (clusterT[:, mslice].T @ style_R[:, nslice])
#
# The aT production is fused with the first n-block's accumulation (n=0,
# m=0..3 accumulate k-outer across 4 PSUM banks) so the PE never drains in
# the prologue. The K=64 style matmuls (tmpLT/tmpR) are row-packed two at a
# time via tile_position into the upper/lower 64 PE rows: the host ships
# cluster/style operands duplicated across partitions 0-63 and 64-127, and
# each packed pair costs one N=512 streaming slot instead of two.
#
# DMA layout: xT and W are pre-arranged on the host partition-major so each
# DMA lands 8-32 KiB contiguous per SBUF partition (big packets). xT streams
# on the Activation HWDGE queue, W + consts + outputs on the Sync queue.

import numpy as np
import ml_dtypes

B, DIN, DOUT, NCL = 8192, 4096, 4096, 64
NCORES = 8
MB = B // NCORES          # batch rows per core
P = 128
NT = 512                  # n tile (dout cols per matmul)
KT = DIN // P             # 32 k tiles
MT = MB // P              # 8 m tiles
NTS = DOUT // NT          # 8 n tiles
FUSED = 4                 # m tiles of n=0 accumulated during the aT prologue
XG = 4                    # k tiles per xT DMA granule
WG = 8                    # k tiles per W DMA granule (n=0 only)

_CACHE = {}
LAST = {}                 # exposes the most recent BassKernelResults for test harnesses


def _build_program():
    import concourse.bacc as bacc
    import concourse.mybir as mybir
    import concourse.tile as tile

    bf16 = mybir.dt.bfloat16
    f32 = mybir.dt.float32

    nc = bacc.Bacc(None, target_bir_lowering=False, debug=False)

    # xT: [granule, partition, k-in-granule, batch]; W: [n, partition, k, nt]
    # cluster/styles arrive duplicated: rows 64-127 = rows 0-63 (row packing).
    xT_d = nc.declare_dram_parameter("xT", [KT // XG, P, XG, MB], bf16, isOutput=False)
    clT_d = nc.declare_dram_parameter("clusterT", [P, MB], bf16, isOutput=False)
    w_d = nc.declare_dram_parameter("weight", [NTS, P, KT, NT], bf16, isOutput=False)
    sL_d = nc.declare_dram_parameter("style_L", [P, DIN], bf16, isOutput=False)
    sR_d = nc.declare_dram_parameter("style_R", [P, DOUT], bf16, isOutput=False)
    out_d = nc.declare_dram_parameter("out", [MB, DOUT], f32, isOutput=True)

    H = NCL  # 64: row-pack halves

    with tile.TileContext(nc) as tc:
        with (
            tc.tile_pool(name="const", bufs=1) as const_pool,
            tc.tile_pool(name="atp", bufs=1) as at_pool,
            tc.tile_pool(name="wp", bufs=2) as w_pool,
            tc.tile_pool(name="xp", bufs=3) as x_pool,
            tc.tile_pool(name="evp", bufs=3) as ev_pool,
            # PSUM budget (8 banks): py 4 x [128,512] (tmpR psum + y
            # accumulators) + pl 2 x [128,1024] (2 banks each) = 8.
            tc.tile_pool(name="pyp", bufs=4, space="PSUM") as py_pool,
            tc.tile_pool(name="plp", bufs=2, space="PSUM") as pl_pool,
        ):
            # ---- constants (clT+sL gate the first tmpLT MMs; sR only gates
            # the tmpR matmuls, which are off the critical path) ----
            clT = const_pool.tile([P, MB], bf16, name="clT")
            sL = const_pool.tile([P, DIN], bf16, name="sL")
            sR = const_pool.tile([P, DOUT], bf16, name="sR")
            # clT+sL gate the first matmul: put them on the otherwise-idle
            # Activation queue so they don't sit behind W0 on the Sync queue.
            nc.scalar.dma_start(clT[:], clT_d[:])
            nc.scalar.dma_start(sL[:], sL_d[:])
            nc.sync.dma_start(sR[:], sR_d[:])

            def tmpr_pair(n, m, psum_src="py"):
                """Row-packed pair: tmpR tiles for (m, m+1) at n, staged to SBUF.

                psum_src="pl" borrows a pl-pool tile (two banks) instead of two
                py slots — required in the fused prologue where all four py
                slots are held by the open accumulators (a py allocation there
                would deadlock against its own epilogue).
                """
                if psum_src == "pl":
                    prp = pl_pool.tile([P, MB], f32, name=f"prf{n}_{m}", tag="pl")
                    pra, prb = prp[:, 0:NT], prp[:, NT:MB]
                else:
                    pra = py_pool.tile([P, NT], f32, name=f"pr{n}_{m}", tag="py")
                    prb = py_pool.tile([P, NT], f32, name=f"pr{n}_{m + 1}", tag="py")
                nc.tensor.matmul(
                    pra[:],
                    clT[:H, m * P:(m + 1) * P],
                    sR[:H, n * NT:(n + 1) * NT],
                    start=True, stop=True, tile_position=(0, 0),
                )
                nc.tensor.matmul(
                    prb[:],
                    clT[H:, (m + 1) * P:(m + 2) * P],
                    sR[H:, n * NT:(n + 1) * NT],
                    start=True, stop=True, tile_position=(H, 0),
                )
                tra = ev_pool.tile([P, NT], f32, name=f"tr{n}_{m}", tag="tr", bufs=6)
                trb = ev_pool.tile([P, NT], f32, name=f"tr{n}_{m + 1}", tag="tr", bufs=6)
                nc.any.tensor_copy(out=tra[:], in_=pra[:])
                nc.any.tensor_copy(out=trb[:], in_=prb[:])
                return tra, trb

            def epilogue(n, m, py, tr):
                ot = ev_pool.tile([P, NT], f32, name=f"ot{n}_{m}", tag="ot")
                nc.vector.tensor_mul(out=ot[:], in0=py[:], in1=tr[:])
                nc.sync.dma_start(
                    out_d[m * P:(m + 1) * P, n * NT:(n + 1) * NT], ot[:]
                )

            # ---- W for n=0, in granules so the first fused MM isn't gated
            # on the whole 4 MiB ----
            w0 = w_pool.tile([P, KT, NT], bf16, name="w0", tag="wbig")
            for j in range(KT // WG):
                nc.sync.dma_start(
                    w0[:, j * WG:(j + 1) * WG, :],
                    w_d[0, :, j * WG:(j + 1) * WG, :],
                )

            # ---- fused prologue: aT production + n0/m0..3 k-outer accumulation ----
            py_f = [
                py_pool.tile([P, NT], f32, name=f"py0_{m}", tag="py")
                for m in range(FUSED)
            ]
            at_tiles = []
            tr_f = []
            for g in range(KT // XG):
                xg = x_pool.tile([P, XG, MB], bf16, name=f"xg{g}", tag="xg")
                nc.scalar.dma_start(xg[:], xT_d[g])
                for j in range(XG):
                    k = g * XG + j
                    # tmpLT: row-packed pair, both batch halves in one slot
                    pl = pl_pool.tile([P, MB], f32, name=f"pl{k}", tag="pl")
                    nc.tensor.matmul(
                        pl[:, 0:NT],
                        sL[:H, k * P:(k + 1) * P],
                        clT[:H, 0:NT],
                        start=True, stop=True, tile_position=(0, 0),
                    )
                    nc.tensor.matmul(
                        pl[:, NT:MB],
                        sL[H:, k * P:(k + 1) * P],
                        clT[H:, NT:MB],
                        start=True, stop=True, tile_position=(H, 0),
                    )
                    at_k = at_pool.tile([P, MB], bf16, name=f"at{k}", tag=f"at{k}")
                    nc.vector.tensor_mul(out=at_k[:], in0=xg[:, j, :], in1=pl[:])
                    at_tiles.append(at_k)
                    for m in range(FUSED):
                        nc.tensor.matmul(
                            py_f[m][:],
                            at_k[:, m * P:(m + 1) * P],
                            w0[:, k, :],
                            start=(k == 0), stop=(k == KT - 1),
                        )
                if g == 1:
                    # tmpR for the fused m tiles; placed here (PE has slack in
                    # the prologue) so it doesn't gate the kernel start on sR
                    tr_f += tmpr_pair(0, 0, psum_src="pl")
                elif g == 2:
                    tr_f += tmpr_pair(0, 2, psum_src="pl")
            for m in range(FUSED):
                epilogue(0, m, py_f[m], tr_f[m])

            # ---- standard m-pair body: two 32-MM groups with the packed tmpR
            # pair injected mid-group (the deep MM pipeline hides its
            # LDWEIGHTS; at a group boundary it costs a full extra slot) ----
            def body_pair(n, m, wn):
                tra = trb = None
                for mm in (m, m + 1):
                    py = py_pool.tile([P, NT], f32, name=f"py{n}_{mm}", tag="py")
                    for k in range(KT):
                        nc.tensor.matmul(
                            py[:],
                            at_tiles[k][:, mm * P:(mm + 1) * P],
                            wn[:, k, :],
                            start=(k == 0), stop=(k == KT - 1),
                        )
                        if mm == m and k == KT // 2:
                            tra, trb = tmpr_pair(n, m)
                    epilogue(n, mm, py, tra if mm == m else trb)

            # rest of n=0
            for m in range(FUSED, MT, 2):
                body_pair(0, m, w0)
            # n = 1..7
            for n in range(1, NTS):
                wn = w_pool.tile([P, KT, NT], bf16, name=f"w{n}", tag="wbig")
                nc.sync.dma_start(wn[:], w_d[n])
                for m in range(0, MT, 2):
                    body_pair(n, m, wn)

    nc.finalize()
    return nc


def _get_program():
    if "nc" not in _CACHE:
        _CACHE["nc"] = _build_program()
    return _CACHE["nc"]


def kernel(x, cluster, weight, style_L, style_R):
    import os

    # The NTFF trace path needs an antenv hook this container lacks; never
    # let a stray BASS_TRACE env take the run down that path.
    os.environ.setdefault("BASS_NEVER_TRACE", "1")
    from concourse.bass_utils import run_bass_kernel_spmd

    nc = _get_program()
    bf16 = ml_dtypes.bfloat16

    # W: [din, dout] -> [n, p, k, nt] partition-major for contiguous DMA
    w_bf = np.asarray(weight, dtype=np.float32).astype(bf16)
    w_r = np.ascontiguousarray(
        w_bf.reshape(KT, P, NTS, NT).transpose(2, 1, 0, 3)
    )
    # styles/cluster duplicated across both 64-row halves for row packing
    sL1 = np.asarray(style_L, dtype=np.float32).astype(bf16)
    sR1 = np.asarray(style_R, dtype=np.float32).astype(bf16)
    sL = np.ascontiguousarray(np.vstack([sL1, sL1]))
    sR = np.ascontiguousarray(np.vstack([sR1, sR1]))

    in_maps = []
    for c in range(NCORES):
        xs = np.asarray(x[c * MB:(c + 1) * MB], dtype=np.float32)
        xT = np.ascontiguousarray(xs.T).astype(bf16)          # [DIN, MB]
        # [din, mb] -> [granule, p, k-in-granule, mb]
        xT_r = np.ascontiguousarray(
            xT.reshape(KT // XG, XG, P, MB).transpose(0, 2, 1, 3)
        )
        clT1 = np.ascontiguousarray(
            np.asarray(cluster[c * MB:(c + 1) * MB], dtype=np.float32).T
        ).astype(bf16)
        clT = np.ascontiguousarray(np.vstack([clT1, clT1]))
        in_maps.append(
            {"xT": xT_r, "clusterT": clT, "weight": w_r, "style_L": sL, "style_R": sR}
        )

    res = run_bass_kernel_spmd(nc, in_maps, list(range(NCORES)))
    LAST["results"] = res
    LAST["in_maps"] = in_maps
    out = np.concatenate(
        [np.asarray(res.results[c]["out"], dtype=np.float32) for c in range(NCORES)],
        axis=0,
    )
    return out



# revision 6
# speedup vs baseline: 1.0255x; 1.0255x over previous
# Bass/Tile TRN2 kernel for nn_Conv1D_style: out = ((x * (cluster@style_L)) @ weight) * (cluster@style_R)
#
# Sharding: data-parallel over the batch dim. Each of the 8 cores gets a
# 1024-row slice of x/cluster and a full (replicated) weight/style_L/style_R.
#
# Per-core plan (M=1024 batch, K=4096 din, N=4096 dout), all matmuls bf16
# with fp32 PSUM accumulation:
#   aT[k] = xT[k] * (style_L[:, kslice].T @ clusterT)  -> bf16, SBUF-resident.
#   y[m,n] = sum_k aT[k][:, mslice].T @ W[k, nslice]   (32 accumulating MMs)
#   out[m,n] = y[m,n] * (clusterT[:, mslice].T @ style_R[:, nslice])
#
# Schedule: the aT production is software-pipelined with the n=0
# accumulation for m0-5 (6 PSUM accumulators, k-outer): per k the PE runs
# [tmpLT pair k][6 fused MMs for k-1], so the vector at-mul for k (1.29us)
# hides under the fused MM group (1.30us) with pl bufs=1 acting as the
# pipeline interlock. All loads stream on the Sync HWDGE queue in exact
# consumption order with fine granules (xT in 2-k-tile granules, W(n=0) in
# 4-k-tile granule tiles, style_L in 8-k-tile granules) so the first PE op
# is gated on ~0.5 MiB, not 13 MiB; w1/w2 prefetch queues strictly after
# the prologue bytes. Output stores (and nothing else) issue on the
# Scalar/Activation queue so stores never delay loads.
#
# The K=64 style matmuls (tmpLT/tmpR) are row-packed two at a time via
# tile_position into the upper/lower 64 PE rows: the host ships
# cluster/style operands duplicated across partitions 0-63 and 64-127, and
# each packed pair costs one ~320ns slot instead of two.

import numpy as np
import ml_dtypes

B, DIN, DOUT, NCL = 8192, 4096, 4096, 64
NCORES = 8
MB = B // NCORES          # batch rows per core
P = 128
NT = 512                  # n tile (dout cols per matmul)
KT = DIN // P             # 32 k tiles
MT = MB // P              # 8 m tiles
NTS = DOUT // NT          # 8 n tiles
FUSED = 6                 # m tiles of n=0 accumulated during the aT prologue
XG = 2                    # k tiles per xT DMA granule (16 granules)
W0G = 4                   # k tiles per W(n=0) granule tile (8 granules)
SLG = 8                   # k tiles per style_L granule (4 granules)

_CACHE = {}
LAST = {}                 # exposes the most recent BassKernelResults for test harnesses


def _build_program():
    import concourse.bacc as bacc
    import concourse.mybir as mybir
    import concourse.tile as tile

    bf16 = mybir.dt.bfloat16
    f32 = mybir.dt.float32

    nc = bacc.Bacc(None, target_bir_lowering=False, debug=False)

    # xT: [granule, partition, k-in-granule, batch]; W: [n, partition, k, nt]
    # cluster/styles arrive duplicated: rows 64-127 = rows 0-63 (row packing).
    xT_d = nc.declare_dram_parameter("xT", [KT // XG, P, XG, MB], bf16, isOutput=False)
    clT_d = nc.declare_dram_parameter("clusterT", [P, MB], bf16, isOutput=False)
    w_d = nc.declare_dram_parameter("weight", [NTS, P, KT, NT], bf16, isOutput=False)
    sL_d = nc.declare_dram_parameter("style_L", [P, DIN], bf16, isOutput=False)
    sR_d = nc.declare_dram_parameter("style_R", [P, DOUT], bf16, isOutput=False)
    out_d = nc.declare_dram_parameter("out", [MB, DOUT], f32, isOutput=True)

    H = NCL  # 64: row-pack halves
    SLW = SLG * P  # cols per sL granule

    with tile.TileContext(nc) as tc:
        with (
            tc.tile_pool(name="const", bufs=1) as const_pool,
            tc.tile_pool(name="atp", bufs=1) as at_pool,
            tc.tile_pool(name="wp", bufs=2) as w_pool,
            tc.tile_pool(name="xp", bufs=4) as x_pool,
            tc.tile_pool(name="evp", bufs=3) as ev_pool,
            # PSUM budget (8 banks): py 6 x [128,512] accumulators + pl
            # 1 x [128,1024] fp32 (2 banks) = 8. tmpR pairs borrow pl.
            tc.tile_pool(name="pyp", bufs=6, space="PSUM") as py_pool,
            tc.tile_pool(name="plp", bufs=1, space="PSUM") as pl_pool,
        ):
            # ---- consts that gate the first PE op: tiny, first in queue ----
            clT = const_pool.tile([P, MB], bf16, name="clT")
            nc.sync.dma_start(clT[:], clT_d[:])
            sL = const_pool.tile([P, DIN], bf16, name="sL")
            nc.sync.dma_start(sL[:, 0:SLW], sL_d[:, 0:SLW])
            sR = const_pool.tile([P, DOUT], bf16, name="sR")

            # W(n=0) granule tiles: independent tiles so their DMAs neither
            # serialize on a shared write-sem nor gate the first MMs.
            w0g = [
                const_pool.tile([P, W0G, NT], bf16, name=f"w0g{j}")
                for j in range(KT // W0G)
            ]

            def w0slice(k):
                return w0g[k // W0G][:, k % W0G, :]

            # prologue DMA emission schedule keyed by k (sync queue order ==
            # consumption order; xg runs 2-3 granules ahead of the at-muls)
            def emit_loads(k):
                if k == 0:
                    for g in (0, 1, 2):
                        xg_dma(g)
                    nc.sync.dma_start(w0g[0][:], w_d[0, :, 0:W0G, :])
                    nc.sync.dma_start(w0g[1][:], w_d[0, :, W0G:2 * W0G, :])
                elif k % 2 == 0:
                    g = k // 2 + 2
                    if g < KT // XG:
                        xg_dma(g)
                    if k % 4 == 0:
                        j = k // 4 + 1
                        if j < KT // W0G:
                            nc.sync.dma_start(
                                w0g[j][:], w_d[0, :, j * W0G:(j + 1) * W0G, :]
                            )
                    if k == 2:
                        nc.sync.dma_start(
                            sL[:, SLW:2 * SLW], sL_d[:, SLW:2 * SLW]
                        )
                    elif k == 6:
                        nc.sync.dma_start(sR[:], sR_d[:])
                    elif k == 10:
                        nc.sync.dma_start(
                            sL[:, 2 * SLW:3 * SLW], sL_d[:, 2 * SLW:3 * SLW]
                        )
                    elif k == 14:
                        nc.sync.dma_start(
                            sL[:, 3 * SLW:4 * SLW], sL_d[:, 3 * SLW:4 * SLW]
                        )

            xg_tiles = {}

            def xg_dma(g):
                xg = x_pool.tile([P, XG, MB], bf16, name=f"xg{g}", tag="xg")
                nc.sync.dma_start(xg[:], xT_d[g])
                xg_tiles[g] = xg

            def tmpr_pair(n, m):
                """Row-packed pair: tmpR tiles for (m, m+1) at n, staged to SBUF.

                Borrows the pl pool tile (two banks); its allocation interlocks
                against the previous tmpLT's at-mul / prior pair's copies.
                """
                prp = pl_pool.tile([P, MB], f32, name=f"prf{n}_{m}", tag="pl")
                pra, prb = prp[:, 0:NT], prp[:, NT:MB]
                nc.tensor.matmul(
                    pra[:],
                    clT[:H, m * P:(m + 1) * P],
                    sR[:H, n * NT:(n + 1) * NT],
                    start=True, stop=True, tile_position=(0, 0),
                )
                nc.tensor.matmul(
                    prb[:],
                    clT[H:, (m + 1) * P:(m + 2) * P],
                    sR[H:, n * NT:(n + 1) * NT],
                    start=True, stop=True, tile_position=(H, 0),
                )
                tra = ev_pool.tile([P, NT], bf16, name=f"tr{n}_{m}", tag="tr", bufs=6)
                trb = ev_pool.tile([P, NT], bf16, name=f"tr{n}_{m + 1}", tag="tr", bufs=6)
                nc.vector.tensor_copy(out=tra[:], in_=pra[:])
                nc.vector.tensor_copy(out=trb[:], in_=prb[:])
                return tra, trb

            def epilogue(n, m, py, tr):
                ot = ev_pool.tile([P, NT], f32, name=f"ot{n}_{m}", tag="ot")
                nc.vector.tensor_mul(out=ot[:], in0=py[:], in1=tr[:])
                nc.scalar.dma_start(
                    out_d[m * P:(m + 1) * P, n * NT:(n + 1) * NT], ot[:]
                )

            # ---- fused prologue: aT production + n0/m0..5 k-outer
            # accumulation, fused MMs emitted one k behind the tmpLT pairs ----
            py_f = [
                py_pool.tile([P, NT], f32, name=f"py0_{m}", tag="py")
                for m in range(FUSED)
            ]
            at_tiles = []
            tr_f = []

            def fused_mms(k):
                for m in range(FUSED):
                    nc.tensor.matmul(
                        py_f[m][:],
                        at_tiles[k][:, m * P:(m + 1) * P],
                        w0slice(k),
                        start=(k == 0), stop=(k == KT - 1),
                    )

            for k in range(KT):
                emit_loads(k)
                # tmpLT: row-packed pair, both batch halves in one pl slot
                pl = pl_pool.tile([P, MB], f32, name=f"pl{k}", tag="pl")
                nc.tensor.matmul(
                    pl[:, 0:NT],
                    sL[:H, k * P:(k + 1) * P],
                    clT[:H, 0:NT],
                    start=True, stop=True, tile_position=(0, 0),
                )
                nc.tensor.matmul(
                    pl[:, NT:MB],
                    sL[H:, k * P:(k + 1) * P],
                    clT[H:, NT:MB],
                    start=True, stop=True, tile_position=(H, 0),
                )
                at_k = at_pool.tile([P, MB], bf16, name=f"at{k}", tag=f"at{k}")
                nc.vector.tensor_mul(
                    out=at_k[:], in0=xg_tiles[k // XG][:, k % XG, :], in1=pl[:]
                )
                at_tiles.append(at_k)
                if k > 0:
                    fused_mms(k - 1)
                # tmpR for the fused m tiles, late in the loop (sR + pl slack)
                if k == 21:
                    tr_f += tmpr_pair(0, 0)
                elif k == 25:
                    tr_f += tmpr_pair(0, 2)
                elif k == 29:
                    tr_f += tmpr_pair(0, 4)
            fused_mms(KT - 1)
            for m in range(FUSED):
                epilogue(0, m, py_f[m], tr_f[m])

            # ---- standard m-pair body: two 32-MM groups with the packed tmpR
            # pair injected mid-group (the deep MM pipeline hides its
            # LDWEIGHTS; at a group boundary it costs a full extra slot) ----
            def body_pair(n, m, wsl):
                tra = trb = None
                for mm in (m, m + 1):
                    py = py_pool.tile([P, NT], f32, name=f"py{n}_{mm}", tag="py")
                    for k in range(KT):
                        nc.tensor.matmul(
                            py[:],
                            at_tiles[k][:, mm * P:(mm + 1) * P],
                            wsl(k),
                            start=(k == 0), stop=(k == KT - 1),
                        )
                        if mm == m and k == KT // 2:
                            tra, trb = tmpr_pair(n, m)
                    epilogue(n, mm, py, tra if mm == m else trb)

            # rest of n=0 (w1 prefetch queues behind the prologue loads)
            w1 = w_pool.tile([P, KT, NT], bf16, name="w1", tag="wbig")
            nc.sync.dma_start(w1[:], w_d[1])
            for m in range(FUSED, MT, 2):
                body_pair(0, m, w0slice)
            # n = 1..7
            wn = w1
            for n in range(1, NTS):
                if n + 1 < NTS:
                    wnext = w_pool.tile([P, KT, NT], bf16, name=f"w{n+1}", tag="wbig")
                    nc.sync.dma_start(wnext[:], w_d[n + 1])
                wcur = wn
                for m in range(0, MT, 2):
                    body_pair(n, m, lambda k, w=wcur: w[:, k, :])
                wn = wnext if n + 1 < NTS else None

    nc.finalize()
    return nc


def _get_program():
    if "nc" not in _CACHE:
        _CACHE["nc"] = _build_program()
    return _CACHE["nc"]


def kernel(x, cluster, weight, style_L, style_R):
    import os

    # The NTFF trace path needs an antenv hook this container lacks; never
    # let a stray BASS_TRACE env take the run down that path.
    os.environ.setdefault("BASS_NEVER_TRACE", "1")
    from concourse.bass_utils import run_bass_kernel_spmd

    nc = _get_program()
    bf16 = ml_dtypes.bfloat16

    # W: [din, dout] -> [n, p, k, nt] partition-major for contiguous DMA
    w_bf = np.asarray(weight, dtype=np.float32).astype(bf16)
    w_r = np.ascontiguousarray(
        w_bf.reshape(KT, P, NTS, NT).transpose(2, 1, 0, 3)
    )
    # styles/cluster duplicated across both 64-row halves for row packing
    sL1 = np.asarray(style_L, dtype=np.float32).astype(bf16)
    sR1 = np.asarray(style_R, dtype=np.float32).astype(bf16)
    sL = np.ascontiguousarray(np.vstack([sL1, sL1]))
    sR = np.ascontiguousarray(np.vstack([sR1, sR1]))

    in_maps = []
    for c in range(NCORES):
        xs = np.asarray(x[c * MB:(c + 1) * MB], dtype=np.float32)
        xT = np.ascontiguousarray(xs.T).astype(bf16)          # [DIN, MB]
        # [din, mb] -> [granule, p, k-in-granule, mb]
        xT_r = np.ascontiguousarray(
            xT.reshape(KT // XG, XG, P, MB).transpose(0, 2, 1, 3)
        )
        clT1 = np.ascontiguousarray(
            np.asarray(cluster[c * MB:(c + 1) * MB], dtype=np.float32).T
        ).astype(bf16)
        clT = np.ascontiguousarray(np.vstack([clT1, clT1]))
        in_maps.append(
            {"xT": xT_r, "clusterT": clT, "weight": w_r, "style_L": sL, "style_R": sR}
        )

    res = run_bass_kernel_spmd(nc, in_maps, list(range(NCORES)))
    LAST["results"] = res
    LAST["in_maps"] = in_maps
    out = np.concatenate(
        [np.asarray(res.results[c]["out"], dtype=np.float32) for c in range(NCORES)],
        axis=0,
    )
    return out
